# revision 12
# baseline (speedup 1.0000x reference)
"""Multi-head self-attention (B=4, S=2048, D=1024, H=16, causal) on 8 TRN2 NeuronCores.

Sharding: tensor-parallel over heads (2 heads/core) for QKV projection + attention,
then AllToAll redistributes attention outputs so the output projection is
token-parallel (1024 tokens/core). No reduction collective needed.

Orientation: everything is computed transposed (feature-major) so that all
matmuls contract over the partition dimension with large free dims:
  Q^T/K^T/V^T [hd, tok] = W^T x^T   (x^T supplied by host)
  S^T [k, q]  = K^T-slice as lhsT, Q^T as rhs  (keys on partitions; causal mask
                added in PSUM via an identity-matmul with precomputed mask tiles)
  attn^T [hd, q] accumulated over key blocks with a fused ones-row in V that
                yields softmax denominators for free
  out [tok, d] = (attn^T chunks as lhsT) @ W_out

Matmuls run in float32r (TF32-like, ~1.2e-4 rel rounding); accumulation fp32.
"""

import numpy as np

B, S, D, H = 4, 2048, 1024, 16
HD = D // H            # 64
CORES = 8
P = 128
TOK = B * S            # 8192 tokens (flattened b,s)
TPC = TOK // CORES     # 1024 tokens per core for out-proj
HPC = H // CORES       # 2 heads per core
QC = 512               # query chunk
NQC = S // QC          # 4 q-chunks per sequence
KB = S // P            # 16 key blocks per sequence
DCH = D // P           # 8 contraction chunks over D
MASKV = -1.0e4         # additive causal mask value (pre-scale); exp underflows to 0

_CACHE = {}


def _build():
    import concourse.mybir as mybir
    import concourse.tile as tile
    from concourse import bacc

    F32 = mybir.dt.float32
    F32R = mybir.dt.float32r
    EXP = mybir.ActivationFunctionType.Exp
    COPY = mybir.ActivationFunctionType.Copy
    MULT = mybir.AluOpType.mult
    ADD = mybir.AluOpType.add

    nc = bacc.Bacc("TRN2", target_bir_lowering=False, debug=False, num_devices=CORES)

    xt = nc.dram_tensor("xt", [D, TOK], F32R, kind="ExternalInput").ap()
    wqkv = nc.dram_tensor("wqkv", [D, 3 * P], F32R, kind="ExternalInput").ap()
    bqkv = nc.dram_tensor("bqkv", [3 * P], F32, kind="ExternalInput").ap()
    wout = nc.dram_tensor("wout", [D, D], F32R, kind="ExternalInput").ap()
    bout = nc.dram_tensor("bout", [D], F32R, kind="ExternalInput").ap()
    masks = nc.dram_tensor("masks", [P, QC // P, QC], F32R, kind="ExternalInput").ap()
    identr = nc.dram_tensor("identr", [P, P], F32R, kind="ExternalInput").ap()
    onesd = nc.dram_tensor("onesd", [P, P], F32R, kind="ExternalInput").ap()
    identf = nc.dram_tensor("identf", [P, HD], F32, kind="ExternalInput").ap()
    out = nc.dram_tensor("out", [TPC, D], F32, kind="ExternalOutput").ap()

    cc_in = nc.dram_tensor("cc_in", [CORES, P, TPC], F32R)
    cc_out = nc.dram_tensor("cc_out", [CORES, P, TPC], F32R)

    DIAG = QC // P  # 4 diagonal key-blocks per q-chunk

    with tile.TileContext(nc) as tc:
        # ================= phase 1: qkv projection + attention =================
        with (
            tc.tile_pool(name="const", bufs=1) as const,
            tc.tile_pool(name="xpool", bufs=2) as xpool,
            tc.tile_pool(name="slab", bufs=2) as slab,
            tc.tile_pool(name="vpool", bufs=2) as vpool,
            tc.tile_pool(name="ppool", bufs=4) as ppool,
            tc.tile_pool(name="epi", bufs=3) as epi,
            tc.tile_pool(name="ps_st", bufs=3, space="PSUM") as ps_st,
            tc.tile_pool(name="ps_ot", bufs=2, space="PSUM") as ps_ot,
        ):
            wq_t = const.tile([P, DCH, 3 * P], F32R)
            nc.sync.dma_start(wq_t[:], wqkv.rearrange("(o p) c -> p o c", p=P))
            bq_t = const.tile([P, 3], F32)
            nc.sync.dma_start(bq_t[:], bqkv.rearrange("(s p) -> p s", p=P))
            mask_t = const.tile([P, DIAG, QC], F32R)
            nc.sync.dma_start(mask_t[:], masks[:])
            idr_t = const.tile([P, P], F32R)
            nc.sync.dma_start(idr_t[:], identr[:])
            idf_t = const.tile([P, HD], F32)
            nc.sync.dma_start(idf_t[:], identf[:])
            ones_t = const.tile([P, P], F32R)
            nc.sync.dma_start(ones_t[:], onesd[:])

            for b in range(B):
                t0 = b * S
                # ---- QKV projection for this batch (tokens t0 .. t0+S) ----
                q2t = slab.tile([P, S], F32R, tag="q2t")   # rows: [head0 64 | head1 64]
                k2t = slab.tile([P, S], F32R, tag="k2t")
                v2t = slab.tile([P, S], F32, tag="v2t")
                for tc_i in range(S // QC):
                    xt_t = xpool.tile([P, DCH, QC], F32R, tag="xt")
                    nc.sync.dma_start(
                        xt_t[:],
                        xt[:, t0 + tc_i * QC : t0 + (tc_i + 1) * QC].rearrange(
                            "(o p) t -> p o t", p=P
                        ),
                    )
                    for s_i, dst in enumerate((q2t, k2t, v2t)):
                        psum3 = ps_st.tile([P, 2, QC], F32, tag="st", name="qkvps")
                        psum = psum3[:, 0, :]
                        for dc in range(DCH):
                            nc.tensor.matmul(
                                psum[:],
                                wq_t[:, dc, s_i * P : (s_i + 1) * P],
                                xt_t[:, dc],
                                start=(dc == 0),
                                stop=(dc == DCH - 1),
                            )
                        nc.vector.tensor_scalar_add(
                            dst[:, tc_i * QC : (tc_i + 1) * QC],
                            psum[:],
                            bq_t[:, s_i : s_i + 1],
                        )

                # ---- transpose V to key-major, append ones row ----
                # v2[h]: [128 (k mod 128), KB, 66]; cols 0:64 = V, col 64 = ones
                v2 = []
                for h in range(HPC):
                    vt = vpool.tile([P, KB, 66], F32R, tag=f"v2_{h}")
                    nc.vector.tensor_copy(vt[:, :, 64:65], ones_t[:, 0:KB, None])
                    v2.append(vt)
                for h in range(HPC):
                    for kb4 in range(KB // 4):
                        pst = ps_st.tile([P, 2, QC], F32, tag="st", name="pst")
                        for j in range(4):
                            kb = kb4 * 4 + j
                            nc.tensor.transpose(
                                pst[:, 0, j * HD : (j + 1) * HD],
                                v2t[h * HD : (h + 1) * HD, kb * P : (kb + 1) * P],
                                idf_t[h * HD : (h + 1) * HD, :],
                            )
                        nc.vector.tensor_copy(
                            v2[h][:, kb4 * 4 : kb4 * 4 + 4, 0:HD],
                            pst[:, 0, 0 : 4 * HD].rearrange("p (a b) -> p a b", b=HD),
                        )

                # ---- causal attention, both heads fused per 2-block group ----
                for qc_i in range(NQC):
                    qsl = slice(qc_i * QC, (qc_i + 1) * QC)
                    nkb = (qc_i + 1) * DIAG  # causal key extent in 128-blocks
                    otp = [ps_ot.tile([P, QC], F32, tag="ot", name=f"ot{h}") for h in range(HPC)]
                    pts = {}
                    for kb2 in range(nkb // 2):
                        for h in range(HPC):
                            hof = h * HD
                            stp = ps_st.tile([P, 2, QC], F32, tag="st", name=f"st{h}")
                            pt = ppool.tile([P, 2, QC], F32R, tag="pt", name=f"pt{h}")
                            pts[(kb2, h)] = (stp, pt)
                        # scores: pack (headA, headB) pairs at row groups 0 / 64
                        for j in range(2):
                            kb = kb2 * 2 + j
                            d = kb - qc_i * DIAG  # >= 0 on diagonal blocks
                            for h in range(HPC):
                                hof = h * HD
                                nc.tensor.matmul(
                                    pts[(kb2, h)][0][:, j, :],
                                    k2t[hof : hof + HD, kb * P : (kb + 1) * P],
                                    q2t[hof : hof + HD, qsl],
                                    start=True,
                                    stop=(d < 0),
                                )
                            if d >= 0:
                                # additive causal mask (-1e4 above diagonal)
                                for h in range(HPC):
                                    nc.tensor.matmul(
                                        pts[(kb2, h)][0][:, j, :],
                                        idr_t[:],
                                        mask_t[:, d, :],
                                        start=False,
                                        stop=True,
                                    )
                        for h in range(HPC):
                            stp, pt = pts[(kb2, h)]
                            nc.scalar.activation(pt[:], stp[:], EXP, scale=0.125)
                            # attn_unnorm^T += V_aug^T P^T  (row 64 = denominators)
                            for j in range(2):
                                kb = kb2 * 2 + j
                                nc.tensor.matmul(
                                    otp[h][0:65, :],
                                    v2[h][:, kb, 0:65],
                                    pt[:, j, :],
                                    start=(kb == 0),
                                    stop=(kb == nkb - 1),
                                )
                    for h in range(HPC):
                        hof = h * HD
                        # epilogue: normalize by denominator (row 64 of otp)
                        den_r = epi.tile([P, QC], F32R, tag="den_r")
                        nc.vector.tensor_copy(den_r[64:65, :], otp[h][64:65, :])
                        dbc3 = ps_st.tile([P, 2, QC], F32, tag="st", name="dbc3")
                        dbc = dbc3[:, 0, :]
                        nc.tensor.matmul(
                            dbc[0:HD, :], ones_t[64:65, 0:HD], den_r[64:65, :],
                            start=True, stop=True,
                        )
                        rden_s = epi.tile([HD, QC], F32, tag="rden_s")
                        nc.vector.reciprocal_approx_fast(rden_s[:], dbc[0:HD, :])
                        attn_s = epi.tile([HD, QC], F32R, tag="attn_s")
                        nc.vector.tensor_tensor(
                            attn_s[:], otp[h][0:HD, :], rden_s[:], MULT
                        )
                        # scatter to A2A input: global tokens t0+qc*QC
                        tg = t0 + qc_i * QC
                        j_core = tg // TPC
                        off = tg % TPC
                        nc.sync.dma_start(
                            cc_in[j_core, hof : hof + HD, off : off + QC], attn_s[:]
                        )

        # ---- exchange: core j receives [128 a-dims from each core, its tokens] ----
        nc.gpsimd.collective_compute(
            "AllToAll",
            mybir.AluOpType.bypass,
            replica_groups=[list(range(CORES))],
            ins=[cc_in.ap().opt()],
            outs=[cc_out.ap().opt()],
        )

        # ================= phase 2: output projection (token-parallel) =========
        with (
            tc.tile_pool(name="opool", bufs=1) as opool,
            tc.tile_pool(name="oev", bufs=3) as oev,
            tc.tile_pool(name="ps_o", bufs=4, space="PSUM") as ps_o,
        ):
            wo_t = opool.tile([P, DCH, D], F32R)
            nc.sync.dma_start(wo_t[:], wout.rearrange("(o p) d -> p o d", p=P))
            bo_t = opool.tile([1, D], F32R)
            nc.sync.dma_start(bo_t[:], bout[None, :])
            ones_o = opool.tile([1, P], F32R)
            nc.sync.dma_start(ones_o[:], onesd[0:1, :])
            at = []
            for j in range(CORES):
                a_t = opool.tile([P, TPC], F32R, name=f"at{j}", tag=f"at_{j}")
                nc.sync.dma_start(a_t[:], cc_out[j])
                at.append(a_t)
            for tb in range(TPC // P):
                for nb in range(D // QC):
                    psum = ps_o.tile([P, QC], F32, tag="omm")
                    for j in range(CORES):
                        nc.tensor.matmul(
                            psum[:],
                            at[j][:, tb * P : (tb + 1) * P],
                            wo_t[:, j, nb * QC : (nb + 1) * QC],
                            start=(j == 0),
                            stop=False,
                        )
                    # bias: rank-1 update ones[tok] x b_out[d]
                    nc.tensor.matmul(
                        psum[:],
                        ones_o[:1, :],
                        bo_t[:1, nb * QC : (nb + 1) * QC],
                        start=False,
                        stop=True,
                    )
                    o_s = oev.tile([P, QC], F32, tag="o_s")
                    nc.vector.tensor_copy(o_s[:], psum[:])
                    nc.sync.dma_start(
                        out[tb * P : (tb + 1) * P, nb * QC : (nb + 1) * QC], o_s[:]
                    )

    nc.compile()
    return nc


def _get_nc():
    if "nc" not in _CACHE:
        _CACHE["nc"] = _build()
    return _CACHE["nc"]


TRACE = False
LAST_RESULT = {}


def kernel(x, W_qkv, b_qkv, W_out, b_out):
    from concourse.bass_utils import run_bass_kernel_spmd

    x = np.asarray(x, dtype=np.float32)
    W_qkv = np.asarray(W_qkv, dtype=np.float32)
    b_qkv = np.asarray(b_qkv, dtype=np.float32)
    W_out = np.asarray(W_out, dtype=np.float32)
    b_out = np.asarray(b_out, dtype=np.float32)

    xt = np.ascontiguousarray(x.reshape(TOK, D).T)  # [D, TOK]

    kk = np.arange(P)[:, None, None]
    dd = np.arange(QC // P)[None, :, None]
    qq = np.arange(QC)[None, None, :]
    masks = np.where(qq >= dd * P + kk, 0.0, MASKV).astype(np.float32)
    ident = np.eye(P, dtype=np.float32)
    ident2 = np.ascontiguousarray(np.tile(np.eye(HD, dtype=np.float32), (2, 1)))

    in_maps = []
    for g in range(CORES):
        c = slice(g * P, (g + 1) * P)
        wq = np.concatenate(
            [W_qkv[:, c], W_qkv[:, D:][:, c], W_qkv[:, 2 * D:][:, c]], axis=1
        )
        bq = np.concatenate([b_qkv[c], b_qkv[D:][c], b_qkv[2 * D:][c]])
        in_maps.append(
            {
                "xt": xt,
                "wqkv": np.ascontiguousarray(wq),
                "bqkv": np.ascontiguousarray(bq),
                "wout": W_out,
                "bout": b_out,
                "masks": masks,
                "identr": ident,
                "onesd": np.ones((P, P), dtype=np.float32),
                "identf": ident2,
            }
        )

    nc = _get_nc()
    res = run_bass_kernel_spmd(
        nc, in_maps, core_ids=list(range(CORES)), trace=TRACE
    )
    LAST_RESULT["res"] = res
    full = np.concatenate([res.results[g]["out"] for g in range(CORES)], axis=0)
    return full.reshape(B, S, D)


# revision 13
# speedup vs baseline: 1.3141x; 1.3141x over previous
"""Multi-head self-attention (B=4, S=2048, D=1024, H=16, causal) on 8 TRN2 NeuronCores.

Sharding: tensor-parallel over heads (2 heads/core) for QKV projection + attention,
then AllToAll redistributes attention outputs so the output projection is
token-parallel (1024 tokens/core). No reduction collective needed.

Orientation: everything is computed transposed (feature-major) so all matmuls
contract over the partition dimension with 512-wide free dims:
  Q^T/K^T/V^T [hd, tok] = W^T x^T    (x^T supplied by host)
  S^T [k, q]  = K^T-block as lhsT, Q^T as rhs (keys on partitions)
  P^T = exp(S^T/8) on ScalarE, causal-masked to 0 by GPSIMD affine_select
  attn^T [hd, q] += V_aug^T P^T      (fused ones-row in V gives denominators)
  out [tok, d] = (attn^T chunks as lhsT) @ W_out

Matmuls in float32r (TF32-like); the attention->out-proj exchange in bf16.
The next batch's QKV projection is interleaved into the attention loop to keep
the PE instruction stream dense (HAM clock-gate stays warm).
"""

import numpy as np

B, S, D, H = 4, 2048, 1024, 16
HD = D // H            # 64
CORES = 8
P = 128
TOK = B * S            # 8192 tokens (flattened b,s)
TPC = TOK // CORES     # 1024 tokens per core for out-proj
HPC = H // CORES       # 2 heads per core
QC = 512               # query chunk
NQC = S // QC          # 4 q-chunks per sequence
KB = S // P            # 16 key blocks per sequence
DCH = D // P           # 8 contraction chunks over D
DIAG = QC // P         # 4 diagonal key-blocks per q-chunk

_CACHE = {}


def _build():
    import concourse.mybir as mybir
    import concourse.tile as tile
    from concourse import bacc

    F32 = mybir.dt.float32
    F32R = mybir.dt.float32r
    BF16 = mybir.dt.bfloat16
    EXP = mybir.ActivationFunctionType.Exp
    MULT = mybir.AluOpType.mult

    nc = bacc.Bacc("TRN2", target_bir_lowering=False, debug=False, num_devices=CORES)

    xt = nc.dram_tensor("xt", [D, TOK], F32R, kind="ExternalInput").ap()
    wqkv = nc.dram_tensor("wqkv", [D, 3 * P], F32R, kind="ExternalInput").ap()
    bqkv = nc.dram_tensor("bqkv", [3 * P], F32, kind="ExternalInput").ap()
    wout = nc.dram_tensor("wout", [D, D], BF16, kind="ExternalInput").ap()
    bout = nc.dram_tensor("bout", [D], BF16, kind="ExternalInput").ap()
    identf = nc.dram_tensor("identf", [P, HD], F32, kind="ExternalInput").ap()
    onesd = nc.dram_tensor("onesd", [P, P], F32R, kind="ExternalInput").ap()
    out = nc.dram_tensor("out", [TPC, D], F32, kind="ExternalOutput").ap()

    # A2A buffers split by q-chunk parity so the first exchange overlaps the tail
    cc_in = [nc.dram_tensor(f"cc_in{i}", [CORES, P, QC], BF16) for i in range(2)]
    cc_out = [nc.dram_tensor(f"cc_out{i}", [CORES, P, QC], BF16) for i in range(2)]

    with tile.TileContext(nc) as tc:
        # ================= phase 1: qkv projection + attention =================
        with (
            tc.tile_pool(name="const", bufs=1) as const,
            tc.tile_pool(name="xpool", bufs=2) as xpool,
            tc.tile_pool(name="slab", bufs=2) as slab,
            tc.tile_pool(name="vpool", bufs=2) as vpool,
            tc.tile_pool(name="ppool", bufs=4) as ppool,
            tc.tile_pool(name="epi", bufs=2) as epi,
            tc.tile_pool(name="ps_st", bufs=3, space="PSUM") as ps_st,
            tc.tile_pool(name="ps_ot", bufs=2, space="PSUM") as ps_ot,
        ):
            wq_t = const.tile([P, DCH, 3 * P], F32R)
            nc.sync.dma_start(wq_t[:], wqkv.rearrange("(o p) c -> p o c", p=P))
            bq_t = const.tile([P, 3], F32)
            nc.sync.dma_start(bq_t[:], bqkv.rearrange("(s p) -> p s", p=P))
            idf_t = const.tile([P, HD], F32)
            nc.sync.dma_start(idf_t[:], identf[:])
            ones_t = const.tile([P, P], F32R)
            nc.sync.dma_start(ones_t[:], onesd[:])

            slabs = {}

            def qkv_tc(b, tc_i):
                """QKV projection matmuls for token chunk tc_i of batch b."""
                q2t, k2t, v2t = slabs[b]
                t0 = b * S
                xt_t = xpool.tile([P, DCH, QC], F32R, tag="xt", name="xt_t")
                nc.sync.dma_start(
                    xt_t[:],
                    xt[:, t0 + tc_i * QC : t0 + (tc_i + 1) * QC].rearrange(
                        "(o p) t -> p o t", p=P
                    ),
                )
                for s_i, dst in enumerate((q2t, k2t, v2t)):
                    psum3 = ps_st.tile([P, 2, QC], F32, tag="st", name="qkvps")
                    psum = psum3[:, 0, :]
                    for dc in range(DCH):
                        nc.tensor.matmul(
                            psum[:],
                            wq_t[:, dc, s_i * P : (s_i + 1) * P],
                            xt_t[:, dc],
                            start=(dc == 0),
                            stop=(dc == DCH - 1),
                        )
                    nc.vector.tensor_scalar_add(
                        dst[:, tc_i * QC : (tc_i + 1) * QC],
                        psum[:],
                        bq_t[:, s_i : s_i + 1],
                    )

            def v_transpose(b):
                """Transpose V^T slab to key-major V_aug tiles (ones row appended)."""
                _, _, v2t = slabs[b]
                v2 = []
                for h in range(HPC):
                    vt = vpool.tile([P, KB, 66], F32R, tag=f"v2_{h}", name=f"v2_{h}")
                    nc.vector.tensor_copy(vt[:, :, 64:65], ones_t[:, 0:KB, None])
                    v2.append(vt)
                for h in range(HPC):
                    for kb4 in range(KB // 4):
                        pst = ps_st.tile([P, 2, QC], F32, tag="st", name="pst")
                        for j in range(4):
                            kb = kb4 * 4 + j
                            nc.tensor.transpose(
                                pst[:, 0, j * HD : (j + 1) * HD],
                                v2t[h * HD : (h + 1) * HD, kb * P : (kb + 1) * P],
                                idf_t[h * HD : (h + 1) * HD, :],
                            )
                        nc.vector.tensor_copy(
                            v2[h][:, kb4 * 4 : kb4 * 4 + 4, 0:HD],
                            pst[:, 0, 0 : 4 * HD].rearrange("p (a b) -> p a b", b=HD),
                        )
                return v2

            def attention_qc(b, qc_i, v2, filler=None):
                """Attention for q-chunk qc_i of batch b, both heads fused.

                filler() is invoked after the first group to interleave
                independent PE work (next batch's QKV) into exp-wait gaps.
                """
                q2t, k2t, _ = slabs[b]
                t0 = b * S
                qsl = slice(qc_i * QC, (qc_i + 1) * QC)
                nkb = (qc_i + 1) * DIAG
                otp = [
                    ps_ot.tile([P, QC], F32, tag="ot", name=f"ot{h}")
                    for h in range(HPC)
                ]
                for kb2 in range(nkb // 2):
                    tiles = []
                    for h in range(HPC):
                        stp = ps_st.tile([P, 2, QC], F32, tag="st", name=f"st{h}")
                        pt = ppool.tile([P, 2, QC], F32R, tag="pt", name=f"pt{h}")
                        tiles.append((stp, pt))
                    # scores: (headA, headB) pairs run concurrently (row groups 0/64)
                    for j in range(2):
                        kb = kb2 * 2 + j
                        for h in range(HPC):
                            hof = h * HD
                            nc.tensor.matmul(
                                tiles[h][0][:, j, :],
                                k2t[hof : hof + HD, kb * P : (kb + 1) * P],
                                q2t[hof : hof + HD, qsl],
                                start=True,
                                stop=True,
                            )
                    for h in range(HPC):
                        stp, pt = tiles[h]
                        nc.scalar.activation(pt[:], stp[:], EXP, scale=0.125)
                        d0 = kb2 * 2 - qc_i * DIAG
                        if d0 + 1 >= 0:  # group touches the causal diagonal
                            nc.gpsimd.affine_select(
                                out=pt[:],
                                in_=pt[:],
                                compare_op=mybir.AluOpType.is_ge,
                                fill=0.0,
                                base=-P * d0,
                                channel_multiplier=-1,
                                pattern=[[-P, 2], [1, QC]],
                            )
                        for j in range(2):
                            kb = kb2 * 2 + j
                            nc.tensor.matmul(
                                otp[h][0:65, :],
                                v2[h][:, kb, 0:65],
                                pt[:, j, :],
                                start=(kb == 0),
                                stop=(kb == nkb - 1),
                            )
                    if filler is not None and kb2 == 0:
                        filler()
                        filler = None
                if filler is not None:
                    filler()
                for h in range(HPC):
                    hof = h * HD
                    # normalize by denominators (row 64): bcast via K=1 matmul
                    den_r = epi.tile([P, QC], F32R, tag="den_r", name="den_r")
                    nc.vector.tensor_copy(den_r[64:65, :], otp[h][64:65, :])
                    dbc3 = ps_st.tile([P, 2, QC], F32, tag="st", name="dbc3")
                    dbc = dbc3[:, 0, :]
                    nc.tensor.matmul(
                        dbc[0:HD, :], ones_t[64:65, 0:HD], den_r[64:65, :],
                        start=True, stop=True,
                    )
                    rden_s = epi.tile([HD, QC], F32, tag="rden_s", name="rden_s")
                    nc.vector.reciprocal_approx_fast(rden_s[:], dbc[0:HD, :])
                    attn_s = epi.tile([HD, QC], BF16, tag="attn_s", name="attn_s")
                    nc.vector.tensor_tensor(attn_s[:], otp[h][0:HD, :], rden_s[:], MULT)
                    # scatter to the parity-split A2A input
                    tg = t0 + qc_i * QC
                    j_core = tg // TPC
                    nc.sync.dma_start(
                        cc_in[qc_i % 2][j_core, hof : hof + HD, :], attn_s[:]
                    )

            # software pipeline: qkv(0) fully, then per batch interleave qkv(b+1)
            slabs[0] = (
                slab.tile([P, S], F32R, tag="q2t", name="q2t0"),
                slab.tile([P, S], F32R, tag="k2t", name="k2t0"),
                slab.tile([P, S], F32, tag="v2t", name="v2t0"),
            )
            for tc_i in range(NQC):
                qkv_tc(0, tc_i)
            v2 = v_transpose(0)

            for b in range(B):
                nb = b + 1
                fillers = []
                if nb < B:
                    slabs[nb] = (
                        slab.tile([P, S], F32R, tag="q2t", name=f"q2t{nb}"),
                        slab.tile([P, S], F32R, tag="k2t", name=f"k2t{nb}"),
                        slab.tile([P, S], F32, tag="v2t", name=f"v2t{nb}"),
                    )
                    fillers = [lambda tc_i=i: qkv_tc(nb, tc_i) for i in range(NQC)]
                for qc_i in range(NQC):
                    attention_qc(
                        b, qc_i, v2,
                        filler=fillers[qc_i] if qc_i < len(fillers) else None,
                    )
                if nb < B:
                    v2 = v_transpose(nb)

        # ---- exchange: core j receives [128 a-dims per core, its tokens] ----
        for i in range(2):
            nc.gpsimd.collective_compute(
                "AllToAll",
                mybir.AluOpType.bypass,
                replica_groups=[list(range(CORES))],
                ins=[cc_in[i].ap().opt()],
                outs=[cc_out[i].ap().opt()],
            )

        # ================= phase 2: output projection (token-parallel) =========
        with (
            tc.tile_pool(name="opool", bufs=1) as opool,
            tc.tile_pool(name="oev", bufs=3) as oev,
            tc.tile_pool(name="ps_o", bufs=4, space="PSUM") as ps_o,
        ):
            wo_t = opool.tile([P, DCH, D], BF16)
            nc.sync.dma_start(wo_t[:], wout.rearrange("(o p) d -> p o d", p=P))
            bo_t = opool.tile([1, D], BF16)
            nc.sync.dma_start(bo_t[:], bout[None, :])
            ones_o = opool.tile([1, P], BF16)
            nc.any.memset(ones_o[:], 1.0)
            at = [[None] * CORES, [None] * CORES]
            for i in range(2):
                for j in range(CORES):
                    a_t = opool.tile([P, QC], BF16, name=f"at{i}_{j}", tag=f"at_{i}_{j}")
                    nc.sync.dma_start(a_t[:], cc_out[i][j])
                    at[i][j] = a_t
            for i in range(2):  # token half (qc parity)
                for tb in range(QC // P):
                    for nb_i in range(D // QC):
                        psum = ps_o.tile([P, QC], F32, tag="omm", name="opsum")
                        for j in range(CORES):
                            nc.tensor.matmul(
                                psum[:],
                                at[i][j][:, tb * P : (tb + 1) * P],
                                wo_t[:, j, nb_i * QC : (nb_i + 1) * QC],
                                start=(j == 0),
                                stop=False,
                            )
                        nc.tensor.matmul(
                            psum[:],
                            ones_o[:1, :],
                            bo_t[:1, nb_i * QC : (nb_i + 1) * QC],
                            start=False,
                            stop=True,
                        )
                        o_s = oev.tile([P, QC], F32, tag="o_s", name="o_s")
                        nc.vector.tensor_copy(o_s[:], psum[:])
                        nc.sync.dma_start(
                            out[
                                i * QC + tb * P : i * QC + (tb + 1) * P,
                                nb_i * QC : (nb_i + 1) * QC,
                            ],
                            o_s[:],
                        )

    nc.compile()
    return nc


def _get_nc():
    if "nc" not in _CACHE:
        _CACHE["nc"] = _build()
    return _CACHE["nc"]


TRACE = False
LAST_RESULT = {}


def kernel(x, W_qkv, b_qkv, W_out, b_out):
    from concourse.bass_utils import run_bass_kernel_spmd
    import ml_dtypes

    x = np.asarray(x, dtype=np.float32)
    W_qkv = np.asarray(W_qkv, dtype=np.float32)
    b_qkv = np.asarray(b_qkv, dtype=np.float32)
    W_out = np.asarray(W_out, dtype=np.float32)
    b_out = np.asarray(b_out, dtype=np.float32)

    xt = np.ascontiguousarray(x.reshape(TOK, D).T)  # [D, TOK]
    ident2 = np.ascontiguousarray(np.tile(np.eye(HD, dtype=np.float32), (2, 1)))
    wout_bf = W_out.astype(ml_dtypes.bfloat16)
    bout_bf = b_out.astype(ml_dtypes.bfloat16)

    in_maps = []
    for g in range(CORES):
        c = slice(g * P, (g + 1) * P)
        wq = np.concatenate(
            [W_qkv[:, c], W_qkv[:, D:][:, c], W_qkv[:, 2 * D:][:, c]], axis=1
        )
        bq = np.concatenate([b_qkv[c], b_qkv[D:][c], b_qkv[2 * D:][c]])
        in_maps.append(
            {
                "xt": xt,
                "wqkv": np.ascontiguousarray(wq),
                "bqkv": np.ascontiguousarray(bq),
                "wout": wout_bf,
                "bout": bout_bf,
                "identf": ident2,
                "onesd": np.ones((P, P), dtype=np.float32),
            }
        )

    nc = _get_nc()
    res = run_bass_kernel_spmd(
        nc, in_maps, core_ids=list(range(CORES)), trace=TRACE
    )
    LAST_RESULT["res"] = res
    full = np.concatenate([res.results[g]["out"] for g in range(CORES)], axis=0)
    return full.reshape(B, S, D)
